# revision 72
# baseline (speedup 1.0000x reference)
"""Trainium2 Bass kernel for nn_EnhancedReflectiveCognitiveGraph (GNN edge-softmax attention).

Math (see reference):
  q/k/v = x @ W{q,k,v}.T + b ; per-edge scores s_e = <q[src_e], k[dest_e]>_head / 4
  softmax over edges sharing src (max-subtraction skipped: scores ~ N(0,1) so
  exp never overflows in fp16 and the weights are mathematically identical)
  agg[dest] += w_e * v[src_e] ; out = agg @ Wo.T + bo

Device strategy (8 cores, node sharding, three SPMD NEFF launches).
All arithmetic is on-device; the host between launches does pure relayout
(gather/permute/pad/concat of device outputs -- same class of work as the
exp permutation, no arithmetic).

  L1 (k proj): each core computes k = x@Wk.T+bk for its node shard.  Host
      assembles the k table and pre-gathers per-edge rows into chunk-slot
      order (k_edgeT), so L2 reads one full-bandwidth linear stream instead
      of per-edge DMA-gather descriptors (256B gather descriptors run at
      half DMA bandwidth and dominated the old kernel).
  L2 (src phase): core c owns edges with src in its shard, laid out in
      128-slot chunks, K chunks per 128-node block, uniform across cores so
      one SPMD program serves all 8.  Nodes are assigned to blocks in
      degree-balanced ("snake") order so no block overflows K chunks.
      Computes q,v from x on the fly.  qeT = q expanded per-edge via PE
      matmuls against a streamed one-hot S (up to 4 chunks per matmul);
      qkT = qeT*k_edgeT elementwise (every 3rd batch drains qe to fp16 via
      ACT to hit the 2x DVE rate); per-head scores via PE matmul against a
      constant head mask; exp on ACT; per-src segment sums via PE matmuls
      with streamed S^T, PSUM-accumulated per block; recip -> u = recip*v
      ("u-table": folds the softmax denominator into the value rows).
  L3 (dest phase): core c owns edges with dest in its shard.  Host
      pre-gathers u rows per edge (u_edgeT) and permutes exp to dest-slot
      order.  wv = exp (broadcast over head dim) * u, split between DVE and
      Pool; per-dest-block scatter-add via PE matmuls with streamed one-hots
      (T^T), PSUM-accumulated per block, then the output projection.  agg is
      complete locally (dest-sharded): no collectives, no racy HBM scatter.
"""

import math
import ml_dtypes
import numpy as np

import concourse.bacc as bacc
import concourse.mybir as mybir
import concourse.tile as tile
from concourse.bass_utils import run_bass_kernel_spmd

# ---------------------------------------------------------------- constants
N = 50000
E = 600000
F = 128
H = 8
Dh = 16
P = 128
C = 8                     # cores
SH = 6272                 # nodes per core, cores 0-6 (49 blocks); core 7: 6096
NB = 49                   # blocks per shard
GB = 32                   # chunks per stream DMA batch
PB = 8                    # chunks per PSUM/compute batch (must divide GB)
F16 = mybir.dt.float16
F8 = mybir.dt.float8e4
F32 = mybir.dt.float32
KG_FP8 = False             # k_edgeT stream dtype (fp8 halves its DMA bytes)
KG_DT = F8 if KG_FP8 else F16


def shard_base(c):
    return c * SH


def shard_len(c):
    return min(N, (c + 1) * SH) - c * SH


# ---------------------------------------------------------------- host prep
class ChunkMap:
    """Uniform chunk structure shared by all cores for one phase: K chunks
    (128 slots each) per node block, block-major."""

    def __init__(self, k):
        self.k = k
        self.nch = NB * k
        self.nslots = self.nch * P

    def block_of(self, c):
        return c // self.k


class CorePlan:
    """Per-core slot layout for one phase.

    `key` = endpoint defining the block (src for L2, dest for L3); `other` =
    endpoint indexing the gather table.  Nodes of the shard are assigned to
    (block, loc) slots in degree-balanced snake order, so every block's edge
    count fits in K chunks.  node_perm[i] = shard-local node of (block, loc)
    = divmod(i, 128)."""

    def __init__(self, cmap, core, key, other, edge_ids):
        base, ln = shard_base(core), shard_len(core)
        self.cmap = cmap
        deg = np.bincount(key - base, minlength=SH)
        deg[ln:] = -1                       # nonexistent nodes last
        order = np.argsort(-deg, kind="stable")
        snake = np.empty(SH, np.int64)
        pos = 0
        for r in range(SH // NB):
            row = order[r * NB:(r + 1) * NB]
            if r % 2:
                row = row[::-1]
            snake[pos:pos + NB] = row
            pos += NB
        # node_perm: index (block*128+loc) -> shard-local node id
        self.node_perm = np.empty(SH, np.int64)
        for b in range(NB):
            self.node_perm[b * P:(b + 1) * P] = snake[b::NB]
        inv = np.empty(SH, np.int64)
        inv[self.node_perm] = np.arange(SH)
        self.node_inv = inv                 # shard-local node -> block*128+loc

        slotid = inv[key - base]            # per edge: block*128+loc
        block, loc = slotid // P, slotid % P
        kk = cmap.k
        self.slot_local = np.full(cmap.nslots, -1, np.int64)
        self.slot_gidx = np.zeros(cmap.nslots, np.int64)
        self.slot_edge = np.full(cmap.nslots, -1, np.int64)
        for b in range(NB):
            m = block == b
            cnt = int(m.sum())
            if cnt == 0:
                continue
            assert cnt <= kk * P, f"block {b} overflow: {cnt} > {kk * P}"
            s0 = b * kk * P
            self.slot_local[s0:s0 + cnt] = loc[m]
            self.slot_gidx[s0:s0 + cnt] = other[m]
            self.slot_edge[s0:s0 + cnt] = edge_ids[m]

    def onehot_stream(self, transposed):
        """[128, nch*128] fp8; chunk c at cols c*128:(c+1)*128.
        transposed=False: S   [key_local, e] ; True: S^T [e, key_local].
        Dummy slots are all-zero columns/rows."""
        cm = self.cmap
        out = np.zeros((P, cm.nch * P), dtype=ml_dtypes.float8_e4m3)
        loc = self.slot_local
        sl_all = np.arange(cm.nslots)
        valid = loc >= 0
        ch = sl_all // P
        row = sl_all % P
        if transposed:
            out[row[valid], ch[valid] * P + loc[valid]] = 1.0
        else:
            out[loc[valid], ch[valid] * P + row[valid]] = 1.0
        return out

    def gather_table(self, table, slot_major=False, fp8=False):
        """Pre-gathered per-slot rows from table [N, F] fp16; dummy slots
        zeroed.  feature-major (L2): [F, nch*slot], partition = feature.
        slot_major (L3): [slot, nch*F], partition = slot-within-chunk.
        fp8: cast to float8_e4m3 (fine for k ~ N(0,1); NOT for u, whose
        dynamic range exceeds fp8)."""
        rows = table[self.slot_gidx]          # [nslots, F]
        rows[self.slot_edge < 0] = 0
        cm = self.cmap
        if slot_major:
            rows = rows.reshape(cm.nch, P, F).transpose(1, 0, 2) \
                .reshape(P, cm.nch * F)
        else:
            rows = rows.T                     # [F, nslots]
        if fp8:
            rows = rows.astype(ml_dtypes.float8_e4m3)
        return np.ascontiguousarray(rows)

    def perm_cols(self, arrT):
        """Permute a [*, SH-padded] node-major array into block/loc order."""
        return np.ascontiguousarray(arrT[:, self.node_perm])

    def unperm_rows(self, arr):
        """Inverse of perm on axis 0 ([SH, *] block/loc-major -> node-major)."""
        return arr[self.node_inv]


def compute_cmap(key, other=None):
    """Uniform chunks-per-block: with snake balancing the per-block edge
    count is ~uniform, so K = ceil(max_core_edges / (NB*P)) + 1 safety is
    enough; verify against the actual balanced assignment instead."""
    k = 1
    for c in range(C):
        base, ln = shard_base(c), shard_len(c)
        m = (key >= base) & (key < base + ln)
        kk = key[m] - base
        deg = np.bincount(kk, minlength=SH)
        deg_sorted = np.sort(deg[:ln])[::-1]
        # snake assignment: block b gets deg_sorted[b::NB] (up to reversal);
        # bound the max block sum by the forward order's worst block
        sums = np.array([deg_sorted[b::NB].sum() for b in range(NB)])
        k = max(k, int(np.ceil(sums.max() / P)))
    return ChunkMap(k)


# ---------------------------------------------------------------- L1: k projection
def build_l1():
    nc = bacc.Bacc("TRN2", target_bir_lowering=False, num_devices=C)
    xT = nc.dram_tensor("xT", [P, NB * P], F16, kind="ExternalInput")
    wkT = nc.dram_tensor("wkT", [P, P], F16, kind="ExternalInput")
    bk_r = nc.dram_tensor("bk_r", [1, P], F16, kind="ExternalInput")
    ones = nc.dram_tensor("ones", [1, P], F16, kind="ExternalInput")
    k_out = nc.dram_tensor("k_out", [P, NB * P], F16, kind="ExternalOutput")
    LB = 7  # blocks per load/store piece

    with tile.TileContext(nc) as tc:
        with tc.tile_pool(name="const", bufs=1) as cpool, \
             tc.tile_pool(name="psum", bufs=4, space="PSUM") as ppool:
            w_sb = cpool.tile([P, P], F16, tag="w")
            nc.scalar.dma_start(w_sb[:], wkT[:])
            b_sb = cpool.tile([1, P], F16, tag="b")
            nc.scalar.dma_start(b_sb[:], bk_r[:])
            ones_sb = cpool.tile([1, P], F16, tag="ones")
            nc.scalar.dma_start(ones_sb[:], ones[:])
            xt = cpool.tile([P, NB * P], F16, tag="xT")
            for p0 in range(0, NB, LB):
                sl = slice(p0 * P, (p0 + LB) * P)
                nc.sync.dma_start(xt[:, sl], xT[:, sl])
            osb = cpool.tile([P, NB * P], F16, tag="osb")
            wr_done, wr_next = [0], [LB]
            for b0 in range(0, NB, 4):
                bn = min(4, NB - b0)
                ps = ppool.tile([P, 4 * P], F32, tag="proj")
                for b in range(b0, b0 + bn):
                    o = (b - b0) * P
                    nc.tensor.matmul(ps[:, o:o + P],
                                     lhsT=xt[:, b * P:(b + 1) * P],
                                     rhs=w_sb[:], start=True, stop=False)
                    nc.tensor.matmul(ps[:, o:o + P], lhsT=ones_sb[:],
                                     rhs=b_sb[:], start=False, stop=True)
                if (b0 // 4) % 2:
                    nc.scalar.copy(osb[:, b0 * P:(b0 + bn) * P],
                                   ps[:, :bn * P])
                else:
                    nc.vector.tensor_copy(osb[:, b0 * P:(b0 + bn) * P],
                                          ps[:, :bn * P])
                if b0 + bn >= wr_next[0] or b0 + bn == NB:
                    sl = slice(wr_done[0] * P, (b0 + bn) * P)
                    nc.sync.dma_start(k_out[:, sl], osb[:, sl])
                    wr_done[0] = b0 + bn
                    wr_next[0] = b0 + bn + LB
    nc.compile()
    return nc


# ---------------------------------------------------------------- L2: src phase
def build_l2(cmap):
    nch, K = cmap.nch, cmap.k
    nc = bacc.Bacc("TRN2", target_bir_lowering=False, num_devices=C)
    xT = nc.dram_tensor("xT", [P, NB * P], F16, kind="ExternalInput")
    wqvT = nc.dram_tensor("wqvT", [P, 2 * P], F16, kind="ExternalInput")
    bqv_r = nc.dram_tensor("bqv_r", [1, 2 * P], F16, kind="ExternalInput")
    ones = nc.dram_tensor("ones", [1, P], F16, kind="ExternalInput")
    hmask = nc.dram_tensor("hmask", [P, H], F8, kind="ExternalInput")
    k_edgeT = nc.dram_tensor("k_edgeT", [P, nch * P], KG_DT,
                             kind="ExternalInput")
    S_st = nc.dram_tensor("S_st", [P, nch * P], F8, kind="ExternalInput")
    ST_st = nc.dram_tensor("ST_st", [P, nch * P], F8, kind="ExternalInput")
    exp_out = nc.dram_tensor("exp_out", [P, nch * H], F16, kind="ExternalOutput")
    u_out = nc.dram_tensor("u_out", [P, NB * P], F16, kind="ExternalOutput")

    with tile.TileContext(nc) as tc:
        with tc.tile_pool(name="resident", bufs=1) as rpool, \
             tc.tile_pool(name="stream", bufs=6) as spool, \
             tc.tile_pool(name="work", bufs=3) as wpool:
            w_sb = rpool.tile([P, 2 * P], F16, tag="w")
            nc.sync.dma_start(w_sb[:], wqvT[:])
            b_sb = rpool.tile([1, 2 * P], F16, tag="b")
            nc.sync.dma_start(b_sb[:], bqv_r[:])
            ones_sb = rpool.tile([1, P], F16, tag="ones")
            nc.sync.dma_start(ones_sb[:], ones[:])
            mask_sb = rpool.tile([P, H], F8, tag="hmask")
            nc.sync.dma_start(mask_sb[:], hmask[:])
            xt = rpool.tile([P, NB * P], F16, tag="xT")
            for p0 in range(0, NB, 7):
                sl = slice(p0 * P, (p0 + 7) * P)
                nc.sync.dma_start(xt[:, sl], xT[:, sl])

            # q, v projections for the shard (q_sb/v_sb: [node_local, b*F])
            q_sb = rpool.tile([P, NB * P], F16, tag="q_sb")
            v_sb = rpool.tile([P, NB * P], F16, tag="v_sb")
            with tc.tile_pool(name="proj_psum", bufs=4, space="PSUM") as ppool:
                for b in range(NB):
                    ps = ppool.tile([P, 2 * P], F32, tag="proj")
                    nc.tensor.matmul(ps[:], lhsT=xt[:, b * P:(b + 1) * P],
                                     rhs=w_sb[:], start=True, stop=False)
                    nc.tensor.matmul(ps[:], lhsT=ones_sb[:], rhs=b_sb[:],
                                     start=False, stop=True)
                    nc.scalar.copy(q_sb[:, b * P:(b + 1) * P], ps[:, 0:P])
                    nc.vector.tensor_copy(v_sb[:, b * P:(b + 1) * P],
                                          ps[:, P:2 * P])

            exp_sb = rpool.tile([P, nch * H], F16, tag="exp_sb")
            seg_sb = rpool.tile([P, NB * H], F32, tag="seg_sb")
            rec = rpool.tile([P, NB * H], F32, tag="rec")
            rrep = rpool.tile([P, NB * P], F16, tag="rrep")
            u_sb = rpool.tile([P, NB * P], F16, tag="u_sb")

            kg_tiles = {}
            s_tiles = {}
            st_tiles = {}
            qpsum_cm = tc.tile_pool(name="qe_psum", bufs=2, space="PSUM")
            spsum_cm = tc.tile_pool(name="sc_psum", bufs=2, space="PSUM")
            gpsum_cm = tc.tile_pool(name="seg_psum", bufs=2, space="PSUM")
            qpsum = qpsum_cm.__enter__()
            spsum = spsum_cm.__enter__()
            gpsum = gpsum_cm.__enter__()

            def stream_tile(tiles, dram, ci, dt, eng=None):
                b0 = ci // GB * GB
                if b0 not in tiles:
                    t = spool.tile([P, GB * P], dt, tag=dram.name,
                                   name=f"strm_{dram.name}_{b0}")
                    n = min(GB, nch - b0) * P
                    (eng or nc.sync).dma_start(t[:, :n],
                                               dram[:, b0 * P:b0 * P + n])
                    tiles[b0] = t
                return tiles[b0][:, (ci - b0) * P:(ci - b0 + 1) * P]

            def stream_span(tiles, dram, ci, cj, dt, eng=None):
                b0 = ci // GB * GB
                assert (cj - 1) // GB * GB == b0
                stream_tile(tiles, dram, ci, dt, eng)
                return tiles[b0][:, (ci - b0) * P:(cj - b0) * P]

            def u_tail(b0, b1):
                """recip -> mask -> broadcast -> u = v*recip -> write, for
                blocks [b0, b1); streamed inside the main loop so the launch
                has no serial tail.  Zero-degree nodes / padding have
                seg == 0 -> 1/0 = inf; mask the reciprocal to 0 there so
                fp16 u stays finite (rows never used)."""
                hsl = slice(b0 * H, b1 * H)
                fsl = slice(b0 * P, b1 * P)
                nb = b1 - b0
                rr = wpool.tile([P, 7 * H], F32, tag="rec_raw")
                nc.vector.reciprocal(rr[:, :nb * H], seg_sb[:, hsl])
                nc.vector.scalar_tensor_tensor(
                    out=rec[:, hsl], in0=seg_sb[:, hsl], scalar=0.0,
                    in1=rr[:, :nb * H],
                    op0=mybir.AluOpType.is_gt, op1=mybir.AluOpType.mult)
                nc.scalar.copy(
                    rrep[:, fsl].rearrange("p (b h d) -> p b h d", h=H, d=Dh),
                    rec[:, hsl].rearrange("p (b h) -> p b h", h=H)
                    [:, :, :, None].broadcast_to([P, nb, H, Dh]))
                nc.gpsimd.tensor_mul(u_sb[:, fsl], v_sb[:, fsl], rrep[:, fsl])
                nc.gpsimd.dma_start(u_out[:, fsl], u_sb[:, fsl])

            def emit_qe(cb0):
                # qeT: expand q rows to slots, up to 4 chunks (512 cols =
                # 1 PSUM bank) per matmul, split at block boundaries.
                # Emitted one batch ahead of its consumers (software
                # pipelining): PE computes batch i+1's qe while DVE
                # multiplies batch i, instead of queueing behind batch i's
                # score/seg matmuls.
                cbn = min(PB, nch - cb0)
                qe = qpsum.tile([P, PB * P], F32, tag="qe")
                ci = cb0
                while ci < cb0 + cbn:
                    blk = cmap.block_of(ci)
                    cj = min(cb0 + cbn, (blk + 1) * K,
                             (ci - cb0) // 4 * 4 + 4 + cb0)
                    nc.tensor.matmul(
                        qe[:, (ci - cb0) * P:(cj - cb0) * P],
                        lhsT=q_sb[:, blk * P:(blk + 1) * P],
                        rhs=stream_span(s_tiles, S_st, ci, cj, F8),
                        start=True, stop=True)
                    ci = cj
                return qe

            seg_ps = None
            qe_next = emit_qe(0)
            for cb0 in range(0, nch, PB):
                cbn = min(PB, nch - cb0)
                bi = cb0 // PB
                qe = qe_next
                if cb0 + PB < nch:
                    qe_next = emit_qe(cb0 + PB)
                # qkT = qeT * k_edgeT (fp16 SBUF out).  Direct-from-PSUM DVE
                # mult runs at 1x; every 3rd batch ACT drains qe to fp16
                # SBUF first so the mult hits the 2x packed rate, balancing
                # DVE against ACT (both stay under the DMA roofline).
                qk = wpool.tile([P, PB * P], F16, tag="qk")
                if bi % 3 == 2 and bi < (nch // PB) - 8:
                    qe16 = wpool.tile([P, PB * P], F16, tag="qe16")
                    nc.scalar.copy(qe16[:, :cbn * P], qe[:, :cbn * P])
                    src_q = qe16
                else:
                    src_q = qe
                ci = cb0
                while ci < cb0 + cbn:
                    cj = min(cb0 + cbn, (ci // GB + 1) * GB)
                    nc.vector.tensor_mul(
                        qk[:, (ci - cb0) * P:(cj - cb0) * P],
                        src_q[:, (ci - cb0) * P:(cj - cb0) * P],
                        stream_span(kg_tiles, k_edgeT, ci, cj, KG_DT,
                                    eng=nc.gpsimd))
                    ci = cj
                # per-head scores via PE against the head mask
                sc = spsum.tile([P, PB * H], F32, tag="sc")
                for ci in range(cb0, cb0 + cbn):
                    nc.tensor.matmul(
                        sc[:, (ci - cb0) * H:(ci - cb0 + 1) * H],
                        lhsT=qk[:, (ci - cb0) * P:(ci - cb0 + 1) * P],
                        rhs=mask_sb[:], start=True, stop=True)
                nc.scalar.activation(
                    out=exp_sb[:, cb0 * H:(cb0 + cbn) * H],
                    in_=sc[:, :cbn * H],
                    func=mybir.ActivationFunctionType.Exp,
                    scale=1.0 / math.sqrt(Dh))
                # segment-sum matmuls, PSUM-accumulated across the whole
                # block (blocks may span two batches); ACT-drained per block
                for ck in range(cb0, cb0 + cbn):
                    blk = cmap.block_of(ck)
                    if ck == blk * K:
                        seg_ps = gpsum.tile([P, H], F32, tag="seg",
                                            name=f"seg_{blk}")
                    nc.tensor.matmul(
                        seg_ps[:],
                        lhsT=stream_tile(st_tiles, ST_st, ck, F8),
                        rhs=exp_sb[:, ck * H:(ck + 1) * H],
                        start=(ck == blk * K), stop=(ck == (blk + 1) * K - 1))
                    if ck == (blk + 1) * K - 1:
                        nc.scalar.copy(seg_sb[:, blk * H:(blk + 1) * H],
                                       seg_ps[:])
                        if blk % 4 == 3:
                            u_tail(blk - 3, blk + 1)
                        elif blk == NB - 1:
                            u_tail(NB - 1, NB)
                if (cb0 // GB != (cb0 + PB) // GB) or cb0 + cbn >= nch:
                    g0 = cb0 // GB * GB
                    nc.scalar.dma_start(
                        exp_out[:, g0 * H:(cb0 + cbn) * H],
                        exp_sb[:, g0 * H:(cb0 + cbn) * H])
            gpsum_cm.__exit__(None, None, None)
            spsum_cm.__exit__(None, None, None)
            qpsum_cm.__exit__(None, None, None)
    nc.compile()
    return nc


# ---------------------------------------------------------------- L3: dest phase
def build_l3(cmap):
    nch, K = cmap.nch, cmap.k
    nc = bacc.Bacc("TRN2", target_bir_lowering=False, num_devices=C)
    u_edgeT = nc.dram_tensor("u_edgeT", [P, nch * P], F16, kind="ExternalInput")
    TT_st = nc.dram_tensor("TT_st", [P, nch * P], F8, kind="ExternalInput")
    exp_in = nc.dram_tensor("exp_in", [P, nch * H], F16, kind="ExternalInput")
    WoT = nc.dram_tensor("WoT", [P, P], F16, kind="ExternalInput")
    bo_r = nc.dram_tensor("bo_r", [1, P], F16, kind="ExternalInput")
    ones = nc.dram_tensor("ones", [1, P], F16, kind="ExternalInput")
    outT = nc.dram_tensor("outT", [P, NB * P], F16, kind="ExternalOutput")

    with tile.TileContext(nc) as tc:
        with tc.tile_pool(name="resident", bufs=1) as rpool, \
             tc.tile_pool(name="stream", bufs=4) as spool, \
             tc.tile_pool(name="work", bufs=3) as wpool, \
             tc.tile_pool(name="agg_psum", bufs=4, space="PSUM") as apsum, \
             tc.tile_pool(name="out_psum", bufs=2, space="PSUM") as opsum:
            exp_sb = rpool.tile([P, nch * H], F16, tag="exp_sb")
            nc.sync.dma_start(exp_sb[:], exp_in[:])
            wo_sb = rpool.tile([P, P], F16, tag="wo")
            nc.sync.dma_start(wo_sb[:], WoT[:])
            bo_sb = rpool.tile([1, P], F16, tag="bo")
            nc.sync.dma_start(bo_sb[:], bo_r[:])
            ones_sb = rpool.tile([1, P], F16, tag="ones")
            nc.sync.dma_start(ones_sb[:], ones[:])
            osb = rpool.tile([P, NB * P], F16, tag="osb")

            ug_tiles = {}
            tt_tiles = {}

            def stream_tile(tiles, dram, ci, dt):
                b0 = ci // GB * GB
                if b0 not in tiles:
                    t = spool.tile([P, GB * P], dt, tag=dram.name,
                                   name=f"strm_{dram.name}_{b0}")
                    n = min(GB, nch - b0) * P
                    nc.sync.dma_start(t[:, :n], dram[:, b0 * P:b0 * P + n])
                    tiles[b0] = t
                return tiles[b0][:, (ci - b0) * P:(ci - b0 + 1) * P]

            def stream_span(tiles, dram, ci, cj, dt):
                b0 = ci // GB * GB
                assert (cj - 1) // GB * GB == b0
                stream_tile(tiles, dram, ci, dt)
                return tiles[b0][:, (ci - b0) * P:(cj - b0) * P]

            # per block: wv = exp (broadcast over d) * u on DVE or Pool,
            # K agg matmuls PSUM-accumulated, one drain, output projection.
            for b in range(NB):
                c0 = b * K
                wv = wpool.tile([P, K * P], F16, tag="wv")
                eng = nc.gpsimd if b % 3 == 2 else nc.vector
                ci = c0
                pi = 0
                while ci < c0 + K:
                    cj = min(c0 + K, (ci // GB + 1) * GB)
                    if b >= NB - 3:
                        # final blocks: split halves across DVE+Pool to
                        # halve the last-block latency
                        cj = min(cj, ci + (K + 1) // 2)
                        eng = nc.vector if pi % 2 == 0 else nc.gpsimd
                    pi += 1
                    eng.tensor_mul(
                        wv[:, (ci - c0) * P:(cj - c0) * P]
                        .rearrange("p (c h d) -> p c h d", h=H, d=Dh),
                        stream_span(ug_tiles, u_edgeT, ci, cj, F16)
                        .rearrange("p (c h d) -> p c h d", h=H, d=Dh),
                        exp_sb[:, ci * H:cj * H]
                        .rearrange("p (c h) -> p c h", h=H)[:, :, :, None]
                        .broadcast_to([P, cj - ci, H, Dh]))
                    ci = cj
                agg_ps = apsum.tile([P, P], F32, tag="agg")
                for ck in range(c0, c0 + K):
                    nc.tensor.matmul(
                        agg_ps[:],
                        lhsT=wv[:, (ck - c0) * P:(ck - c0 + 1) * P],
                        rhs=stream_tile(tt_tiles, TT_st, ck, F8),
                        start=(ck == c0), stop=(ck == c0 + K - 1))
                agg16 = wpool.tile([P, P], F16, tag="agg16")
                nc.scalar.copy(agg16[:], agg_ps[:])
                ops = opsum.tile([P, P], F32, tag="outp")
                nc.tensor.matmul(ops[:], lhsT=wo_sb[:], rhs=agg16[:],
                                 start=True, stop=False)
                nc.tensor.matmul(ops[:], lhsT=bo_sb[:], rhs=ones_sb[:],
                                 start=False, stop=True)
                nc.scalar.copy(osb[:, b * P:(b + 1) * P], ops[:])
                if b % 7 == 6:
                    sl = slice((b - 6) * P, (b + 1) * P)
                    nc.scalar.dma_start(outT[:, sl], osb[:, sl])
    nc.compile()
    return nc


# ---------------------------------------------------------------- orchestration
def kernel(node_features, edge_index, Wq, bq, Wk, bk, Wv, bv, Wo, bo):
    node_features = np.asarray(node_features, np.float32)
    edge_index = np.asarray(edge_index)
    src, dst = edge_index[0].astype(np.int64), edge_index[1].astype(np.int64)
    x16 = node_features.astype(np.float16)
    w16 = {k: np.asarray(v, np.float32).astype(np.float16)
           for k, v in (("Wq", Wq), ("Wk", Wk), ("Wv", Wv), ("Wo", Wo))}
    b16 = {k: np.asarray(v, np.float32).astype(np.float16)
           for k, v in (("bq", bq), ("bk", bk), ("bv", bv), ("bo", bo))}
    ones_row = np.ones((1, P), np.float16)
    hmask = np.zeros((P, H), dtype=ml_dtypes.float8_e4m3)
    for h in range(H):
        hmask[h * Dh:(h + 1) * Dh, h] = 1.0
    cores = list(range(C))

    xts = []
    for c in cores:
        base, ln = shard_base(c), shard_len(c)
        xt = np.zeros((P, NB * P), np.float16)
        xt[:, :ln] = x16[base:base + ln].T
        xts.append(xt)

    # ---------------- L1: k table
    nc1 = build_l1()
    in1 = [dict(xT=xts[c], wkT=w16["Wk"].T.copy(),
                bk_r=b16["bk"].reshape(1, P), ones=ones_row)
           for c in cores]
    r1 = run_bass_kernel_spmd(nc1, in1, core_ids=cores)

    k_full = np.zeros((N, P), np.float16)
    for c in cores:
        base, ln = shard_base(c), shard_len(c)
        # k_out[p, b*P+f] is node base+b*128+p, feature f
        ksh = r1.results[c]["k_out"].reshape(P, NB, P).transpose(1, 0, 2) \
            .reshape(NB * P, P)
        k_full[base:base + ln] = ksh[:ln]

    # ---------------- L2: src phase
    eids = np.arange(E, dtype=np.int64)
    cmap2 = compute_cmap(src)
    plans2 = []
    for c in cores:
        base, ln = shard_base(c), shard_len(c)
        m = (src >= base) & (src < base + ln)
        plans2.append(CorePlan(cmap2, c, src[m], dst[m], eids[m]))

    nc2 = build_l2(cmap2)
    in2 = []
    for c in cores:
        pl = plans2[c]
        in2.append(dict(
            xT=pl.perm_cols(xts[c]),
            wqvT=np.concatenate([w16["Wq"].T, w16["Wv"].T], axis=1).copy(),
            bqv_r=np.concatenate([b16["bq"], b16["bv"]]).reshape(1, 2 * P),
            ones=ones_row, hmask=hmask,
            k_edgeT=pl.gather_table(k_full, fp8=KG_FP8),
            S_st=pl.onehot_stream(False), ST_st=pl.onehot_stream(True)))
    r2 = run_bass_kernel_spmd(nc2, in2, core_ids=cores)

    exp_edge = np.zeros((E, H), np.float16)
    u_full = np.zeros((N, P), np.float16)
    for c in cores:
        pl = plans2[c]
        exp_flat = r2.results[c]["exp_out"].reshape(P, cmap2.nch, H) \
            .transpose(1, 0, 2).reshape(cmap2.nslots, H)
        real = pl.slot_edge >= 0
        exp_edge[pl.slot_edge[real]] = exp_flat[real]
        base, ln = shard_base(c), shard_len(c)
        # u_out[p, b*P+f]: (block,loc)-ordered rows -> unpermute to node order
        ush = r2.results[c]["u_out"].reshape(P, NB, P).transpose(1, 0, 2) \
            .reshape(NB * P, P)
        u_full[base:base + ln] = pl.unperm_rows(ush)[:ln]

    # ---------------- L3: dest phase
    cmap3 = compute_cmap(dst)
    plans3 = []
    for c in cores:
        base, ln = shard_base(c), shard_len(c)
        m = (dst >= base) & (dst < base + ln)
        plans3.append(CorePlan(cmap3, c, dst[m], src[m], eids[m]))

    nc3 = build_l3(cmap3)
    in3 = []
    for c in cores:
        pl = plans3[c]
        exp_slots = np.zeros((cmap3.nslots, H), np.float16)
        real = pl.slot_edge >= 0
        exp_slots[real] = exp_edge[pl.slot_edge[real]]
        exp_in = exp_slots.reshape(cmap3.nch, P, H).transpose(1, 0, 2) \
            .reshape(P, cmap3.nch * H)
        in3.append(dict(
            u_edgeT=pl.gather_table(u_full, slot_major=True),
            TT_st=pl.onehot_stream(True),
            exp_in=np.ascontiguousarray(exp_in), WoT=w16["Wo"].T.copy(),
            bo_r=b16["bo"].reshape(1, P), ones=ones_row))
    r3 = run_bass_kernel_spmd(nc3, in3, core_ids=cores)

    out = np.zeros((N, F), np.float32)
    for c in cores:
        pl = plans3[c]
        base, ln = shard_base(c), shard_len(c)
        osh = r3.results[c]["outT"].reshape(P, NB, P)  # [f, b, loc]
        osh = osh.transpose(1, 2, 0).reshape(NB * P, P)
        out[base:base + ln] = pl.unperm_rows(osh)[:ln].astype(np.float32)
    return out
